# revision 24
# baseline (speedup 1.0000x reference)
"""Distributed Trainium2 kernel for the gated-adapter attention module.

Head-parallel tensor parallelism over 8 NeuronCores (4 heads each).
Structure (v6):
  * x is read f32 once, cast to bf16 on the Scalar engine, and transposed
    on the PE via real matmuls against identity (keeps HAM warm; no
    SWDGE/HWDGE DMA contention),
  * wq/wk/wv are staged+transposed the same way during the prologue,
  * wo is cast f32->bf16 by SWDGE cast-DMAs released one-by-one during
    the attention phase (so they never starve latency-critical loads),
    then phase 3 reads wo^T tiles via single-ring xbar transposes with
    the first d-quarter prefetched before the AllToAll,
  * V stays resident in SBUF; causal mask generated on-chip,
  * softmax sum and its partition-broadcast are fused into a single
    accumulating ones-matmul,
  * attention output is written straight into the AllToAll input layout.
Compute bf16, f32 PSUM accumulation.
"""

import sys

sys.path.insert(0, "/opt/trn_rl_repo")

import numpy as np

import concourse.bass as bass
import concourse.mybir as mybir
import concourse.tile as tile
from concourse import bacc, bass_utils
from concourse.bass import ds, ts
from concourse.masks import make_identity

N_CORES = 8
B, S, D = 2, 2048, 4096
H = 32
HD = 128                      # head dim
H_LOC = H // N_CORES          # 4 heads per core
CH = H_LOC * HD               # 512 local channels
TOK = B * S                   # 4096 tokens
NK = D // 128                 # 32 contraction tiles
AL = 10                       # adapter length
TPC = TOK // N_CORES          # 512 tokens per core after AllToAll
NQC = S // 512                # 4 query chunks per sequence
SCALE = 1.0 / float(np.sqrt(HD))
BF = mybir.dt.bfloat16
F32 = mybir.dt.float32
EXP = mybir.ActivationFunctionType.Exp
COPY = mybir.ActivationFunctionType.Copy
TANH = mybir.ActivationFunctionType.Tanh
MULT = mybir.AluOpType.mult
ADD = mybir.AluOpType.add


def build():
    nc = bacc.Bacc("TRN2", target_bir_lowering=False, debug=False,
                   num_devices=N_CORES)
    x = nc.dram_tensor("x", [TOK, D], F32, kind="ExternalInput")
    wq = nc.dram_tensor("wq", [CH, D], F32, kind="ExternalInput")
    wk = nc.dram_tensor("wk", [CH, D], F32, kind="ExternalInput")
    wv = nc.dram_tensor("wv", [CH, D], F32, kind="ExternalInput")
    wo = nc.dram_tensor("wo", [D, D], F32, kind="ExternalInput")
    gate = nc.dram_tensor("gate", [1, H_LOC], F32, kind="ExternalInput")
    adapter = nc.dram_tensor("adapter", [AL, D], F32, kind="ExternalInput")
    fcos = nc.dram_tensor("fcos", [S, HD // 2], F32, kind="ExternalInput")
    fsin = nc.dram_tensor("fsin", [S, HD // 2], F32, kind="ExternalInput")
    out = nc.dram_tensor("out", [TPC, D], F32, kind="ExternalOutput")

    with tile.TileContext(nc) as tc:
        with tc.tile_pool(name="dram", bufs=1, space="DRAM") as dram, \
             tc.tile_pool(name="persist", bufs=1) as persist:
            wob_d = dram.tile([D, D], BF, tag="wob_d")
            qn_ds = [dram.tile([S, CH], BF, tag=f"qn{b}", name=f"qn{b}")
                     for b in range(B)]
            kn_ds = [dram.tile([S, CH], BF, tag=f"kn{b}", name=f"kn{b}")
                     for b in range(B)]
            a2a_in_h = [dram.tile([N_CORES, HD, TPC], BF, tag=f"a2ai{h}",
                                  name=f"a2ai{h}") for h in range(H_LOC)]
            a2a_out_h = [dram.tile([N_CORES, HD, TPC], BF, tag=f"a2ao{h}",
                                   name=f"a2ao{h}") for h in range(H_LOC)]

            ident = persist.tile([128, 128], BF, tag="ident")
            make_identity(nc, ident[:])
            # single causal diagonal block, k on partitions, q on free dim:
            # keep 0 where q >= k else -1e30
            maskT = persist.tile([128, 128], BF, tag="maskT")
            nc.gpsimd.memset(maskT[:], 0.0)
            nc.gpsimd.affine_select(
                out=maskT[:], in_=maskT[:],
                compare_op=mybir.AluOpType.is_ge, fill=-1e30,
                base=0, pattern=[[1, 128]], channel_multiplier=-1)
            thr_dummy = persist.tile([128, 1], BF, tag="thr_dummy")

            ones = persist.tile([128, 128], BF, tag="ones")
            nc.vector.memset(ones[:], 1.0)
            g_sb = persist.tile([128, H_LOC], F32, tag="g_sb")
            g_in = persist.tile([128, H_LOC], F32, tag="g_in")
            nc.scalar.dma_start(g_in[:], gate.ap().partition_broadcast(128))
            nc.scalar.activation(g_sb[:], g_in[:], TANH)
            cs_all = persist.tile([128, S // 128, HD // 2], BF, tag="cs_all")
            sn_all = persist.tile([128, S // 128, HD // 2], BF, tag="sn_all")
            a_kT = persist.tile([128, H_LOC, AL], BF, tag="a_kT")
            a_v = persist.tile([AL, H_LOC, HD], BF, tag="a_v")
            v_all = persist.tile([128, TOK // 128, CH], BF, tag="v_all")

            # ================= phase 1: weights + QKV =================
            with tc.tile_pool(name="wph", bufs=1) as wph, \
                 tc.tile_pool(name="pst", bufs=2, space="PSUM") as pst, \
                 tc.tile_pool(name="psb", bufs=2, space="PSUM") as psb:
                aT = persist.tile([128, NK, AL], BF, tag="aT")
                wTs = [wph.tile([128, NK, CH], BF, tag=f"wT{p_i}",
                                name=f"wT{p_i}") for p_i in range(3)]
                # wq/wk/wv: load f32, cast, transpose on PE via real matmuls
                with tc.tile_pool(name="stg", bufs=2) as stg:
                    for tbl, dst in ((fcos, cs_all), (fsin, sn_all)):
                        cs_f = stg.tile([128, S // 128, HD // 2], F32,
                                        tag="cs_f")
                        nc.sync.dma_start(
                            cs_f[:],
                            tbl.ap().rearrange("(pb p) f -> p pb f", p=128))
                        nc.vector.tensor_copy(dst[:], cs_f[:])
                    for p_i, wt in ((0, wq), (1, wk), (2, wv)):
                        for cs in range(H_LOC):
                            wb = stg.tile([128, D], BF, tag="wb")
                            for hf in range(2):
                                wf = stg.tile([128, D // 2], F32, tag="wf")
                                nc.scalar.dma_start(
                                    wf[:],
                                    wt.ap()[ts(cs, 128), ts(hf, D // 2)])
                                nc.vector.tensor_copy(wb[:, ts(hf, D // 2)],
                                                      wf[:])
                            for dt in range(NK):
                                tps = pst.tile([128, 128], F32, tag="tps")
                                nc.tensor.matmul(tps[:],
                                                 lhsT=wb[:, ts(dt, 128)],
                                                 rhs=ident[:],
                                                 start=True, stop=True)
                                nc.vector.tensor_copy(
                                    wTs[p_i][:, dt, ts(cs, 128)], tps[:])
                    # adapter^T [128 dim, AL] tiles
                    ab = stg.tile([AL, D], BF, tag="ab", bufs=1)
                    for hf in range(4):
                        af = stg.tile([AL, D // 4], F32, tag="af")
                        nc.scalar.dma_start(af[:],
                                            adapter.ap()[:, ts(hf, D // 4)])
                        nc.vector.tensor_copy(ab[:, ts(hf, D // 4)], af[:])
                    for dt in range(NK):
                        aps = pst.tile([128, 128], F32, tag="tps")
                        nc.tensor.matmul(aps[:, :AL], lhsT=ab[:, ts(dt, 128)],
                                         rhs=ident[:AL, :AL],
                                         start=True, stop=True)
                        nc.vector.tensor_copy(aT[:, dt, :], aps[:, :AL])

                # main QKV over 128-token chunks: x read f32 once, cast on
                # ACT, transposed on PE, then three N=512 matmul streams
                with tc.tile_pool(name="run", bufs=2) as st:
                    for tstr in range(TOK // 128):
                        b_i = tstr // (S // 128)
                        srow = (tstr % (S // 128)) * 128
                        xn = st.tile([128, D], BF, tag="xn")
                        for hf in range(2):
                            xf = st.tile([128, D // 2], F32, tag="xf")
                            nc.scalar.dma_start(
                                xf[:], x.ap()[ds(tstr * 128, 128),
                                              ts(hf, D // 2)])
                            nc.scalar.activation(xn[:, ts(hf, D // 2)],
                                                 xf[:], COPY)
                        xT = st.tile([128, NK, 128], BF, tag="xT")
                        for dt in range(NK):
                            tps = pst.tile([128, 128], F32, tag="tps")
                            nc.tensor.matmul(tps[:], lhsT=xn[:, ts(dt, 128)],
                                             rhs=ident[:],
                                             start=True, stop=True)
                            nc.vector.tensor_copy(xT[:, dt, :], tps[:])
                        pps = [psb.tile([128, CH], F32, tag=f"pp{pn}",
                                        name=f"pp{pn}") for pn in "qkv"]
                        for dt in range(NK):
                            for p_i in range(3):
                                nc.tensor.matmul(
                                    pps[p_i][:],
                                    lhsT=xT[:, dt, :],
                                    rhs=wTs[p_i][:, dt, :],
                                    start=(dt == 0), stop=(dt == NK - 1))
                        # v: cast into resident SBUF tile
                        nc.vector.tensor_copy(v_all[:, tstr, :], pps[2][:])
                        # q, k: RoPE then store natural
                        csb = cs_all[:, srow // 128, :]
                        ssb = sn_all[:, srow // 128, :]
                        for p_i, dstl in ((0, qn_ds), (1, kn_ds)):
                            rp = st.tile([128, CH], BF, tag=f"rp{p_i}",
                                         name=f"rp{p_i}")
                            for h in range(H_LOC):
                                pv2 = pps[p_i][:, ts(h, HD)].rearrange(
                                    "p (i two) -> p two i", two=2)
                                rv = rp[:, ts(h, HD)].rearrange(
                                    "p (i two) -> p two i", two=2)
                                a0, b0 = pv2[:, 0, :], pv2[:, 1, :]
                                t1 = st.tile([128, HD // 2], F32, tag="t1")
                                t2 = st.tile([128, HD // 2], F32, tag="t2")
                                nc.vector.tensor_mul(t1[:], a0, csb)
                                nc.vector.tensor_mul(t2[:], b0, ssb)
                                nc.vector.tensor_sub(rv[:, 0, :],
                                                     t1[:], t2[:])
                                nc.vector.tensor_mul(t1[:], a0, ssb)
                                nc.vector.tensor_mul(t2[:], b0, csb)
                                nc.vector.tensor_add(rv[:, 1, :],
                                                     t1[:], t2[:])
                            nc.sync.dma_start(
                                dstl[b_i][ds(srow, 128), :], rp[:])

                # adapter k/v projections (needed only from phase 2 on)
                for cs in range(H_LOC):
                    pk = psb.tile([128, CH], F32, tag="ppq")
                    for dt in range(NK):
                        nc.tensor.matmul(pk[:, :AL],
                                         lhsT=wTs[1][:, dt, ts(cs, 128)],
                                         rhs=aT[:, dt, :], start=(dt == 0),
                                         stop=(dt == NK - 1))
                    nc.vector.tensor_copy(a_kT[:, cs, :], pk[:, :AL])
                pv = psb.tile([128, CH], F32, tag="ppq")
                for dt in range(NK):
                    nc.tensor.matmul(pv[:AL, :], lhsT=aT[:, dt, :],
                                     rhs=wTs[2][:, dt, :], start=(dt == 0),
                                     stop=(dt == NK - 1))
                for cs in range(H_LOC):
                    nc.vector.tensor_copy(a_v[:, cs, :], pv[:AL, ts(cs, 128)])

            # ========= phase 2: attention (h-major, chunked AllToAll) ====
            # head-major order so each local head's AllToAll chunk fires as
            # soon as both batches are done, overlapping collectives with
            # the remaining attention compute
            order = [(h, b) for h in range(H_LOC) for b in range(B)]
            seq = [sc * H_LOC + hh for hh in range(H_LOC)
                   for sc in range(N_CORES)]
            with tc.tile_pool(name="wo_sb", bufs=4) as wsb, \
                 tc.tile_pool(name="wo0", bufs=1) as w0p, \
                 tc.tile_pool(name="of", bufs=1) as ofp:
                oTf = ofp.tile([128, NK, TPC], BF, tag="oTf")
                wot0 = w0p.tile([128, 16, 1024], BF, tag="wot0")
                with tc.tile_pool(name="at", bufs=3) as at, \
                     tc.tile_pool(name="att", bufs=3) as att, \
                     tc.tile_pool(name="ps_st", bufs=3,
                                  space="PSUM") as ps_st, \
                     tc.tile_pool(name="ps_ac", bufs=1,
                                  space="PSUM") as ps_ac:
                    def _bh_loads(b_i, h):
                        qTb = at.tile([128, S], BF, tag="qTb", name="qTb")
                        nc.sync.dma_start_transpose(
                            qTb[:], qn_ds[b_i][:, ts(h, HD)])
                        kTb = at.tile([128, S], BF, tag="kTb", name="kTb")
                        nc.sync.dma_start_transpose(
                            kTb[:], kn_ds[b_i][:, ts(h, HD)])
                        return qTb, kTb

                    def _issue_coll(hh):
                        nc.gpsimd.collective_compute(
                            "AllToAll", mybir.AluOpType.bypass,
                            replica_groups=[list(range(N_CORES))],
                            ins=[a2a_in_h[hh].opt()],
                            outs=[a2a_out_h[hh].opt()])
                        for sc in range(N_CORES):
                            nc.scalar.dma_start(
                                oTf[:, sc * H_LOC + hh, :],
                                a2a_out_h[hh][sc])

                    tiles = {0: _bh_loads(order[0][1], order[0][0]),
                             1: _bh_loads(order[1][1], order[1][0])}
                    for pos, (h, b_i) in enumerate(order):
                        if pos + 2 < len(order):
                            tiles[pos + 2] = _bh_loads(order[pos + 2][1],
                                                       order[pos + 2][0])
                        # collectives are issued only after every attention
                        # transpose is already in flight (pos 5+), so the
                        # transpose-vs-collective serialization never blocks
                        # the q/k loads; data for h0/h1 is long since ready
                        if pos == 5:
                            _issue_coll(0)
                            _issue_coll(1)
                        qTb, kTb = tiles.pop(pos)
                        for qc in range(NQC):
                            nkt = (qc + 1) * 4
                            stb = att.tile([128, S // 128, 512], BF,
                                           tag="stb", bufs=2)
                            for kt in range(nkt):
                                sps = ps_st.tile([128, 512], F32, tag="sps")
                                nc.tensor.matmul(sps[:],
                                                 lhsT=kTb[:, ts(kt, 128)],
                                                 rhs=qTb[:, ts(qc, 512)],
                                                 start=True, stop=True)
                                if kt // 4 == qc:
                                    off = (kt % 4) * 128
                                    if off > 0:
                                        nc.vector.memset(
                                            stb[:, kt, ds(0, off)], 0.0)
                                    sd = att.tile([128, 128], F32, tag="sd")
                                    nc.vector.scalar_tensor_tensor(
                                        sd[:], sps[:, ds(off, 128)], SCALE,
                                        maskT[:], op0=MULT, op1=ADD)
                                    nc.scalar.activation(
                                        stb[:, kt, ds(off, 128)], sd[:], EXP)
                                    if off + 128 < 512:
                                        nc.scalar.activation(
                                            stb[:, kt,
                                                ds(off + 128, 384 - off)],
                                            sps[:, ds(off + 128, 384 - off)],
                                            EXP, scale=SCALE)
                                else:
                                    nc.scalar.activation(stb[:, kt, :],
                                                         sps[:], EXP,
                                                         scale=SCALE)
                            # adapter scores [AL, 512]
                            spa = ps_st.tile([128, 512], F32, tag="sps")
                            nc.tensor.matmul(spa[:AL, :], lhsT=a_kT[:, h, :],
                                             rhs=qTb[:, ts(qc, 512)],
                                             start=True, stop=True)
                            pab = att.tile([AL, 512], BF, tag="pab")
                            nc.scalar.activation(pab[:], spa[:AL, :], EXP,
                                                 scale=SCALE)
                            # fused sum+broadcast: ones[128,128] lhsT makes
                            # every output row the column sum
                            bc_ps = ps_ac.tile([128, 512], F32, tag="bc_ps")
                            for kt in range(nkt):
                                nc.tensor.matmul(bc_ps[:], lhsT=ones[:],
                                                 rhs=stb[:, kt, :],
                                                 start=(kt == 0),
                                                 stop=(kt == nkt - 1))
                            bca_ps = ps_ac.tile([128, 512], F32,
                                                tag="bca_ps")
                            nc.tensor.matmul(bca_ps[:], lhsT=ones[:AL, :],
                                             rhs=pab[:], start=True,
                                             stop=True)
                            # PV accumulation: oT [128 d, 512 q]
                            o_ps = ps_ac.tile([128, 512], F32, tag="o_ps",
                                              bufs=2)
                            for kt in range(nkt):
                                nc.tensor.matmul(
                                    o_ps[:],
                                    lhsT=v_all[:, b_i * (S // 128) + kt,
                                               ts(h, HD)],
                                    rhs=stb[:, kt, :],
                                    start=(kt == 0), stop=(kt == nkt - 1))
                            oa_ps = ps_ac.tile([128, 512], F32, tag="oa_ps")
                            nc.tensor.matmul(oa_ps[:], lhsT=a_v[:, h, :],
                                             rhs=pab[:], start=True,
                                             stop=True)
                            # combine: o = o/s_main + tanh(g)*oa/s_adapt
                            rb = att.tile([128, 512], F32, tag="rb",
                                          bufs=2)
                            nc.vector.reciprocal_approx_fast(rb[:], bc_ps[:])
                            rba = att.tile([128, 512], F32, tag="rba",
                                           bufs=2)
                            nc.vector.reciprocal_approx_fast(rba[:],
                                                             bca_ps[:])
                            t3 = att.tile([128, 512], F32, tag="t3",
                                          bufs=2)
                            nc.vector.tensor_mul(t3[:], o_ps[:], rb[:])
                            t4 = att.tile([128, 512], F32, tag="t4",
                                          bufs=2)
                            nc.vector.scalar_tensor_tensor(
                                t4[:], rba[:], g_sb[:, ds(h, 1)], oa_ps[:],
                                op0=MULT, op1=MULT)
                            ob = att.tile([128, 512], BF, tag="ob")
                            nc.vector.tensor_add(ob[:], t3[:], t4[:])
                            nc.scalar.dma_start(
                                a2a_in_h[h][b_i * NQC + qc][:, :], ob[:])
                            if qc in (1, 3):
                                # release a 256-row wo cast chunk: small
                                # SWDGE bursts that cannot be hoisted
                                # earlier (WAW via the dummy write)
                                j = pos * 2 + qc // 2
                                nc.gpsimd.tensor_copy(thr_dummy[:],
                                                      ob[:, 0:1])
                                nc.gpsimd.dma_start(
                                    wob_d[ds(j * 256, 1), ds(0, 1)],
                                    thr_dummy[0:1, 0:1])
                                nc.gpsimd.dma_start(
                                    wob_d[ds(j * 256, 256), :],
                                    wo.ap()[ds(j * 256, 256), :])
                        if pos == 5:
                            _issue_coll(2)
                        elif pos == 7:
                            _issue_coll(3)
                        if pos == 3:
                            # first two wo row-quarters are cast by now:
                            # prefetch dp=0 wo^T tiles for the first half
                            # of the matmul sequence during attention
                            for idx in range(16):
                                et = seq[idx]
                                nc.sync.dma_start_transpose(
                                    wot0[:, idx, :],
                                    wob_d[ds(0, 1024), ts(et, 128)])

                # ============= phase 3: output projection =============
                with tc.tile_pool(name="wo_ps", bufs=1,
                                  space="PSUM") as wps, \
                     tc.tile_pool(name="wo_sb2", bufs=10) as wsb2:
                    # 4 passes over d; 8 psum banks = 4 tt x 2 d2; ets
                    # ordered by AllToAll-chunk arrival
                    for dp in range(4):
                        yps = [wps.tile([128, 512], F32, tag=f"yp{i}",
                                        name=f"yp{i}") for i in range(8)]
                        for idx, et in enumerate(seq):
                            if dp == 0 and idx < 16:
                                wot = wot0[:, idx, :]
                            else:
                                wotn = wsb2.tile([128, 1024], BF,
                                                 tag="wot")
                                nc.sync.dma_start_transpose(
                                    wotn[:],
                                    wob_d[ds(dp * 1024, 1024), ts(et, 128)])
                                wot = wotn[:]
                            for tt in range(TPC // 128):
                                for d2 in range(2):
                                    nc.tensor.matmul(
                                        yps[tt * 2 + d2][:],
                                        lhsT=oTf[:, et, ts(tt, 128)],
                                        rhs=wot[:, ts(d2, 512)],
                                        start=(idx == 0), stop=(idx == 31))
                        for tt in range(TPC // 128):
                            for d2 in range(2):
                                yb = wsb.tile([128, 512], F32, tag="yb")
                                nc.vector.tensor_copy(yb[:],
                                                      yps[tt * 2 + d2][:])
                                nc.scalar.dma_start(
                                    out.ap()[ts(tt, 128),
                                             ds(dp * 1024 + d2 * 512, 512)],
                                    yb[:])
    nc.compile()
    return nc


_NC_CACHE = None


def kernel(x, wq, wk, wv, wo, gate, adapter, freqs_cos, freqs_sin, mask,
           start_pos=0, **_unused):
    global _NC_CACHE
    if _NC_CACHE is None:
        _NC_CACHE = build()
    nc = _NC_CACHE
    xf = np.ascontiguousarray(np.asarray(x, np.float32).reshape(TOK, D))
    g = np.asarray(gate, np.float32).reshape(H)
    in_maps = []
    for r in range(N_CORES):
        sl = slice(r * CH, (r + 1) * CH)
        in_maps.append({
            "x": xf,
            "wq": np.ascontiguousarray(np.asarray(wq, np.float32)[sl]),
            "wk": np.ascontiguousarray(np.asarray(wk, np.float32)[sl]),
            "wv": np.ascontiguousarray(np.asarray(wv, np.float32)[sl]),
            "wo": np.ascontiguousarray(np.asarray(wo, np.float32)),
            "gate": np.ascontiguousarray(
                g[r * H_LOC:(r + 1) * H_LOC].reshape(1, H_LOC)),
            "adapter": np.ascontiguousarray(
                np.asarray(adapter, np.float32).reshape(AL, D)),
            "fcos": np.ascontiguousarray(np.asarray(freqs_cos, np.float32)),
            "fsin": np.ascontiguousarray(np.asarray(freqs_sin, np.float32)),
        })
    res = bass_utils.run_bass_kernel_spmd(nc, in_maps,
                                          core_ids=list(range(N_CORES)))
    y = np.concatenate([res.results[r]["out"] for r in range(N_CORES)], axis=0)
    return y.reshape(B, S, D)


if __name__ == "__main__":
    nc = build()
    print("compiled ok, instrs:",
          sum(len(bb.instructions) for f in nc.m.functions for bb in f.blocks))
